# revision 7
# baseline (speedup 1.0000x reference)
"""Trainium2 Bass kernel for nn_Attention_84756884619871.

Causal multi-head attention (B=2, S=2048, D=2048, H=16, Dh=128) with RoPE,
fused QKV projection and output projection.

Sharding (8 NeuronCores): data-parallel over batch (2 groups) x
tensor-parallel over heads (4 cores/group, 4 heads each).  Each core:
  - single pass over x^T in 512-column chunks computes q^T,k^T (RoPE
    applied via one ScalarE PSUM evacuation + three bf16 DVE ops) AND v
  - flash-style attention in score-transposed space (p^T[t,s]); softmax
    denominator via ones-vector matmul; no max-subtraction (scores are
    small: exp is safe in fp32)
  - attention runs per 512-column s-quarter, interleaved with the
    projection chunks (quarter q is emitted right after x-chunk q), so
    each of the four AllGathers launches as early as possible and
    overlaps the remaining projection/attention compute
  - the output projections for quarters 0-2 run while the final
    AllGather is in flight; only quarter 3's 64 matmuls trail it
  - DMA queue discipline: weight/x loads + agin stores on the SP HWDGE
    ring, output stores on the ACT ring, ctxg gathers on the Pool SWDGE
    queue, so no long-latency wait ever head-blocks another queue
All matmuls run in bf16 at N=512 (full PE rate, one PSUM bank each).
Host assembles the full [2,2048,2048] output from the 8 column slices.
"""

import numpy as np
import ml_dtypes

import concourse.bass as bass
import concourse.tile as tile
import concourse.mybir as mybir
from concourse import bacc
from concourse.tile import add_dep_helper
from contextlib import ExitStack

F32 = mybir.dt.float32
F32R = mybir.dt.bfloat16  # matmul operand dtype
AF = mybir.ActivationFunctionType

D = 2048
S = 2048
NCORES = 8
TPDEG = 4          # tensor-parallel group size (heads)
HLOC = 4           # heads per core
DH = 128
SCALE = float(1.0 / np.sqrt(DH))
NQ = 4             # s-quarters for attention / collectives
QW = S // NQ       # 512

_STATE: dict = {}
_PHASES: list = []


def _build(reps=1):
    _PHASES.clear()
    nc = bacc.Bacc("TRN2", target_bir_lowering=False, debug=False, num_devices=NCORES)
    xT = nc.dram_tensor("xT", [D, S], F32R, kind="ExternalInput")
    wqk = nc.dram_tensor("wqk", [D, 1024], F32R, kind="ExternalInput")
    wv = nc.dram_tensor("wv", [D, 512], F32R, kind="ExternalInput")
    wo = nc.dram_tensor("wo", [D, 512], F32R, kind="ExternalInput")
    cosT = nc.dram_tensor("cosT", [128, S], F32R, kind="ExternalInput")
    sinTs = nc.dram_tensor("sinTs", [128, S], F32R, kind="ExternalInput")
    trimask = nc.dram_tensor("trimask", [128, 128], F32R, kind="ExternalInput")
    out = nc.dram_tensor("out", [S, 512], F32, kind="ExternalOutput")

    xT3 = xT.ap().rearrange("(ko ki) s -> ki ko s", ki=128)
    wqk3 = wqk.ap().rearrange("(ko ki) c -> ki ko c", ki=128)
    wv3 = wv.ap().rearrange("(ko ki) c -> ki ko c", ki=128)
    wo3 = wo.ap().rearrange("(ko ki) c -> ki ko c", ki=128)

    with tile.TileContext(nc) as tc, ExitStack() as top:
        # ---- persistent constants -----------------------------------------
        per = top.enter_context(tc.tile_pool(name="persist", bufs=1))
        mask_sb = per.tile([128, 128], F32R, name="mask")
        nc.sync.dma_start(mask_sb[:], trimask.ap())
        ones_c0 = per.tile([128, 128], F32, name="ones_c0")
        nc.vector.memset(ones_c0[:], 1.0)
        ones_sq = per.tile([128, 128], F32R, name="ones_sq")
        nc.vector.tensor_copy(ones_sq[:], ones_c0[:])

        dram = top.enter_context(tc.tile_pool(name="dram", bufs=1, space="DRAM"))
        agin = [dram.tile([HLOC * 128, QW], F32R, name=f"agin{q}") for q in range(NQ)]
        agout = [dram.tile([D, QW], F32R, name=f"agout{q}") for q in range(NQ)]

        qkv = top.enter_context(ExitStack())
        qk_pool = qkv.enter_context(tc.tile_pool(name="qkpool", bufs=1))
        qrot = [qk_pool.tile([128, S], F32R, name=f"qrot{h}") for h in range(HLOC)]
        krot = [qk_pool.tile([128, S], F32R, name=f"krot{h}") for h in range(HLOC)]
        v_pool = qkv.enter_context(tc.tile_pool(name="vpool", bufs=1))
        vsb = [v_pool.tile([128, 512], F32R, name=f"v{j}") for j in range(16)]

        wo_pool = top.enter_context(tc.tile_pool(name="wop", bufs=1))
        wo_sb = wo_pool.tile([128, 16, 512], F32R, name="wo_sb")

        for _rep in range(reps):
            _emit_rep(nc, tc, _rep, xT3, wqk3, wv3, wo3, cosT, sinTs, out,
                      mask_sb, ones_sq, agin, agout,
                      qrot, krot, vsb, wo_sb)

    _PHASES.append(('end', nc.next_id()))
    nc.compile()
    return nc


def _emit_rep(nc, tc, rep, xT3, wqk3, wv3, wo3, cosT, sinTs, out,
              mask_sb, ones_sq, agin, agout,
              qrot, krot, vsb, wo_sb):
    """One full kernel body: A-chunks (projection+RoPE+v) interleaved with
    attention quarters so each AllGather launches as early as possible;
    quarter 3 is woven with the earlier quarters' output projections."""
    last_pe = [None]

    with ExitStack() as outer:
        p_pool = outer.enter_context(tc.tile_pool(name="pp", bufs=4))
        misc = outer.enter_context(tc.tile_pool(name="miscb", bufs=2))

        def attn_quarter(sq, sc_ps, ctx_ps, l_ps, after_head=None):
            _PHASES.append((f'attn{sq}', nc.next_id()))
            for h in range(HLOC):
                ctx = ctx_ps.tile([128, QW], F32, tag="ctx", name=f"ctx{sq}_{h}")
                lps = l_ps.tile([128, QW], F32, tag="l", name=f"l{sq}_{h}")
                jmax = 4 * sq + 4
                for j in range(jmax):
                    dj = j - 4 * sq
                    c0 = max(0, 128 * dj)
                    sct = sc_ps.tile([128, QW], F32, tag="scps",
                                     name=f"sc{sq}_{h}_{j}")
                    nc.tensor.matmul(
                        sct[:, c0:QW], krot[h][:, 128 * j:128 * j + 128],
                        qrot[h][:, QW * sq + c0:QW * sq + QW],
                        start=True, stop=True,
                    )
                    p_t = p_pool.tile([128, QW], F32R, tag="p",
                                      name=f"p{sq}_{h}_{j}")
                    nc.scalar.activation(p_t[:, c0:QW], sct[:, c0:QW],
                                         AF.Exp, scale=SCALE)
                    if dj >= 0:
                        dsl = slice(c0, c0 + 128)
                        nc.vector.tensor_mul(p_t[:, dsl], p_t[:, dsl], mask_sb[:])
                    last = (j == jmax - 1)
                    pv_mm = nc.tensor.matmul(
                        ctx[:, c0:QW], vsb[j][:, 128 * h:128 * h + 128],
                        p_t[:, c0:QW], start=(j == 0), stop=last,
                        skip_group_check=True,
                    )
                    last_pe[0] = pv_mm
                    nc.tensor.matmul(
                        lps[:, c0:QW], ones_sq[:], p_t[:, c0:QW],
                        start=(j == 0), stop=last, skip_group_check=True,
                    )
                # normalize: ctxn = ctx * (1/l); the ones-128 stationary
                # already broadcast l across all partitions, so a plain
                # elementwise reciprocal+mul suffices (no PE broadcast)
                linv = misc.tile([128, QW], F32, tag="linv", name=f"li{sq}_{h}")
                nc.vector.reciprocal_approx_fast(out=linv[:], in_=lps[:])
                ctxn = misc.tile([128, QW], F32R, tag="ctxn", name=f"cn{sq}_{h}")
                nc.vector.tensor_mul(ctxn[:], ctx[:], linv[:])
                # SP-ring HWDGE: every A-phase load is emitted before any
                # attention, so the SP queue is drained by now; this keeps
                # the ctxn cross-engine wait off the ACT sequencer, which is
                # the bottleneck engine inside quarter 3
                nc.sync.dma_start(agin[sq][128 * h:128 * h + 128, :], ctxn[:])
                if after_head is not None:
                    after_head(h)
            _PHASES.append((f'AG{sq}', nc.next_id()))
            nc.gpsimd.collective_compute(
                "AllGather", mybir.AluOpType.bypass,
                ins=[agin[sq][:]], outs=[agout[sq][:]],
                replica_groups=[[0, 1, 2, 3], [4, 5, 6, 7]],
            )

        with ExitStack() as st:
            wqk_pool = st.enter_context(tc.tile_pool(name="wqkp", bufs=1))
            wqk_sb = wqk_pool.tile([128, 16, 1024], F32R, name="wqk_sb")
            wv_pool = st.enter_context(tc.tile_pool(name="wvp", bufs=1))
            wv_sb = wv_pool.tile([128, 16, 512], F32R, name="wv_sb")
            cs_pool = st.enter_context(tc.tile_pool(name="csp", bufs=1))
            cos_sb = cs_pool.tile([128, S], F32R, name="cos_sb")
            sin_sb = cs_pool.tile([128, S], F32R, name="sin_sb")
            xt_pool = st.enter_context(tc.tile_pool(name="xtp", bufs=3))
            xt_tiles = [None] * NQ

            def xt_dma(sc, part=None):
                if xt_tiles[sc] is None:
                    xt_tiles[sc] = xt_pool.tile([128, 16, QW], F32R, tag="xt",
                                                name=f"xt{sc}")
                t = xt_tiles[sc]
                ssl = slice(QW * sc, QW * sc + QW)
                if part is None:
                    nc.sync.dma_start(t[:], xT3[:, :, ssl])
                else:
                    nc.sync.dma_start(t[:, 4 * part:4 * part + 4, :],
                                      xT3[:, 4 * part:4 * part + 4, ssl])

            # startup-critical DMA order: first weight group + first x slab
            # land before the rest so the first matmuls can issue early.
            # wv/cos/sin go on the ACT ring (idle until the c-phase stores)
            # so the SP ring only carries the wqk+x bytes the first q/k
            # matmul chains are actually waiting on.
            nc.sync.dma_start(wqk_sb[:, 0:4, :], wqk3[:, 0:4, :])
            xt_dma(0, 0)
            for g in range(1, 4):
                nc.sync.dma_start(wqk_sb[:, 4 * g:4 * g + 4, :],
                                  wqk3[:, 4 * g:4 * g + 4, :])
                xt_dma(0, g)
            for g in range(4):
                nc.scalar.dma_start(wv_sb[:, 4 * g:4 * g + 4, :],
                                    wv3[:, 4 * g:4 * g + 4, :])
            nc.scalar.dma_start(cos_sb[:], cosT.ap())
            nc.scalar.dma_start(sin_sb[:], sinTs.ap())
            xt_dma(1)
            xt_dma(2)
            if rep == 0:
                for g in range(4):
                    nc.scalar.dma_start(wo_sb[:, 4 * g:4 * g + 4, :],
                                        wo3[:, 4 * g:4 * g + 4, :])

            rope_pool = st.enter_context(tc.tile_pool(name="ropep", bufs=2))
            ps_qk = st.enter_context(tc.tile_pool(name="psqk", bufs=2,
                                                  space="PSUM"))
            ps_v = st.enter_context(tc.tile_pool(name="psv", bufs=2,
                                                 space="PSUM"))
            sc_e = st.enter_context(tc.tile_pool(name="scpse", bufs=2,
                                                 space="PSUM"))
            ctx_e = st.enter_context(tc.tile_pool(name="ctxpse", bufs=1,
                                                  space="PSUM"))
            l_e = st.enter_context(tc.tile_pool(name="lpse", bufs=1,
                                                space="PSUM"))

            def a_chunk(sc):
                _PHASES.append((f'A:sc{sc}', nc.next_id()))
                sl = slice(QW * sc, QW * sc + QW)
                xt_c = xt_tiles[sc]
                for m in range(8):
                    pq = ps_qk.tile([128, QW], F32, tag="pq", name=f"pq{sc}_{m}")
                    for ko in range(16):
                        nc.tensor.matmul(pq[:],
                                         wqk_sb[:, ko, 128 * m:128 * m + 128],
                                         xt_c[:, ko, :],
                                         start=(ko == 0), stop=(ko == 15))
                    pqb = rope_pool.tile([128, QW], F32R, tag="pqb",
                                         name=f"pqb{sc}_{m}")
                    nc.scalar.copy(pqb[:], pq[:])
                    t1 = rope_pool.tile([128, QW], F32R, tag="t1",
                                        name=f"t1_{sc}_{m}")
                    nc.vector.tensor_mul(t1[:], pqb[:], cos_sb[:, sl])
                    t2 = rope_pool.tile([128, QW], F32R, tag="t2",
                                        name=f"t2_{sc}_{m}")
                    # rotate-half reads must come from PSUM: a partition-base
                    # mismatch between two SBUF operands is rejected by the
                    # BIR verifier, but PSUM+SBUF operands carry independent
                    # base-partition offsets
                    nc.vector.tensor_mul(t2[0:64, :], pq[64:128, :],
                                         sin_sb[0:64, sl])
                    nc.vector.tensor_mul(t2[64:128, :], pq[0:64, :],
                                         sin_sb[64:128, sl])
                    dest = (qrot[m] if m < 4 else krot[m - 4])[:, sl]
                    nc.vector.tensor_add(dest, t1[:], t2[:])
                for u in range(4):
                    j = 4 * sc + u
                    pv = ps_v.tile([128, 512], F32, tag="pv", name=f"pv{j}")
                    for ko in range(16):
                        nc.tensor.matmul(
                            pv[:], xt_c[:, ko, 128 * u:128 * u + 128],
                            wv_sb[:, ko, :], start=(ko == 0), stop=(ko == 15),
                        )
                    nc.scalar.copy(vsb[j][:], pv[:])

            a_chunk(0)
            xt_dma(3)
            attn_quarter(0, sc_e, ctx_e, l_e)
            a_chunk(1)
            attn_quarter(1, sc_e, ctx_e, l_e)
            a_chunk(2)
            attn_quarter(2, sc_e, ctx_e, l_e)
            a_chunk(3)

        # ---- quarter 3 + phase C woven (A pools released) -----------------
        with ExitStack() as st:
            cg_pool = st.enter_context(tc.tile_pool(name="cgp", bufs=4))
            osb_pool = st.enter_context(tc.tile_pool(name="osbp", bufs=2))
            ps_o = st.enter_context(tc.tile_pool(name="pso", bufs=2,
                                                 space="PSUM"))
            sc_l = st.enter_context(tc.tile_pool(name="scpsl", bufs=3,
                                                 space="PSUM"))
            ctx_l = st.enter_context(tc.tile_pool(name="ctxpsl", bufs=2,
                                                  space="PSUM"))
            l_l = st.enter_context(tc.tile_pool(name="lpsl", bufs=1,
                                                space="PSUM"))
            ctxg = [None] * NQ

            def cg_load(sq):
                _PHASES.append((f'cg{sq}', nc.next_id()))
                cg = cg_pool.tile([128, 16, QW], F32R, tag="cg",
                                  name=f"ctxg{sq}")
                ag3 = agout[sq].rearrange("(ko ki) s -> ki ko s", ki=128)
                # SWDGE queue: must not head-block the HWDGE rings.
                # Split per ko-group so the c-block matmuls for group g
                # only wait on their own slice (pipelines the gather load
                # with the output projection instead of serializing).
                for g4 in range(4):
                    nc.gpsimd.dma_start(cg[:, 4 * g4:4 * g4 + 4, :],
                                        ag3[:, 4 * g4:4 * g4 + 4, :])
                ctxg[sq] = cg

            def c_block(sq, mms):
                _PHASES.append((f'c{sq}.{mms[0]}', nc.next_id()))
                anchor = last_pe[0]
                for mm in mms:
                    m = 4 * sq + mm
                    po = ps_o.tile([128, 512], F32, tag="pso", name=f"po{m}")
                    for ko in range(16):
                        o_mm = nc.tensor.matmul(
                            po[:], ctxg[sq][:, ko, 128 * mm:128 * mm + 128],
                            wo_sb[:, ko, :], start=(ko == 0), stop=(ko == 15),
                        )
                        if ko == 0 and anchor is not None:
                            # keep the scheduler from hoisting the output
                            # projection ahead of attention compute that
                            # does not depend on the AllGather
                            add_dep_helper(o_mm.ins, anchor.ins, sync=True,
                                           reason="order C after attention")
                        anchor = o_mm
                        last_pe[0] = o_mm
                    osb = osb_pool.tile([128, 512], F32, tag="osb",
                                        name=f"osb{m}")
                    nc.scalar.copy(osb[:], po[:])
                    # ACT ring: producer (scalar.copy) is on the same engine,
                    # so the store issues with no cross-engine wait and stays
                    # clear of the Pool queue that carries the cg loads
                    nc.scalar.dma_start(out.ap()[128 * m:128 * m + 128, :],
                                        osb[:])

            # cg3 is emitted late: its wait on the final AllGather must not
            # head-block the Pool queue in front of earlier quarters' traffic
            cg_load(0)
            cg_load(1)
            cg_load(2)

            attn_quarter(3, sc_l, ctx_l, l_l)
            c_block(0, [0, 1, 2, 3])
            c_block(1, [0, 1, 2, 3])
            c_block(2, [0, 1, 2, 3])
            cg_load(3)
            c_block(3, [0, 1, 2, 3])


def _get_runner():
    """Build (once) a persistent jitted SPMD executor for the kernel program."""
    if "runner" in _STATE:
        return _STATE["runner"]
    import jax
    from jax.sharding import Mesh, PartitionSpec
    from jax.experimental.shard_map import shard_map
    from concourse import bass2jax

    nc = _build()
    bass2jax.install_neuronx_cc_hook()

    in_names, out_names, out_avals = [], [], []
    for alloc in nc.m.functions[0].allocations:
        if not isinstance(alloc, mybir.MemoryLocationSet):
            continue
        name = alloc.memorylocations[0].name
        pname = nc.partition_id_tensor.name if nc.partition_id_tensor else None
        if alloc.kind == "ExternalInput":
            if name != pname:
                in_names.append(name)
        elif alloc.kind == "ExternalOutput":
            out_names.append(name)
            out_avals.append(
                jax.core.ShapedArray(tuple(alloc.tensor_shape),
                                     mybir.dt.np(alloc.dtype))
            )
    n_params = len(in_names)
    all_in = list(in_names) + list(out_names)
    pname = nc.partition_id_tensor.name if nc.partition_id_tensor else None
    if pname is not None:
        all_in.append(pname)

    def _body(*args):
        operands = list(args)
        if pname is not None:
            operands.append(bass2jax.partition_id_tensor())
        outs = bass2jax._bass_exec_p.bind(
            *operands,
            out_avals=tuple(out_avals),
            in_names=tuple(all_in),
            out_names=tuple(out_names),
            lowering_input_output_aliases=(),
            sim_require_finite=False,
            sim_require_nnan=False,
            nc=nc,
        )
        return tuple(outs)

    devices = jax.devices()[:NCORES]
    mesh = Mesh(np.asarray(devices), ("core",))
    specs = (PartitionSpec("core"),)
    sharded = jax.jit(
        shard_map(
            _body, mesh=mesh,
            in_specs=specs * (n_params + len(out_names)),
            out_specs=specs * len(out_names),
            check_rep=False,
        ),
        keep_unused=True,
    )
    runner = {
        "fn": sharded, "in_names": in_names, "out_names": out_names,
        "out_avals": out_avals, "n_params": n_params,
    }
    _STATE["runner"] = runner
    return runner


def _prep_inputs(x, cos, sin, w_qkv, w_o):
    """Host-side sharding: per-core input dict list."""
    x = np.asarray(x, dtype=np.float32)
    cos = np.asarray(cos, dtype=np.float32)
    sin = np.asarray(sin, dtype=np.float32)
    w_qkv = np.asarray(w_qkv, dtype=np.float32)
    w_o = np.asarray(w_o, dtype=np.float32)

    bf = ml_dtypes.bfloat16
    cosT = np.ascontiguousarray(cos.T).astype(bf)           # [128, S]
    sinT = sin.T
    sinTs = np.ascontiguousarray(
        np.concatenate([-sinT[0:64], sinT[64:128]], axis=0)).astype(bf)
    pp, ff = np.meshgrid(np.arange(128), np.arange(128), indexing="ij")
    trimask = (pp <= ff).astype(np.float32)                 # t <= s

    in_maps = []
    for c in range(NCORES):
        b, tp = c // TPDEG, c % TPDEG
        cs = 512 * tp
        xT = np.ascontiguousarray(x[b].T)                   # [D, S]
        wq = w_qkv[:, cs:cs + 512]
        wk = w_qkv[:, D + cs:D + cs + 512]
        wqk = np.ascontiguousarray(np.concatenate([wq, wk], axis=1))
        wvs = np.ascontiguousarray(w_qkv[:, 2 * D + cs:2 * D + cs + 512])
        wos = np.ascontiguousarray(w_o[:, cs:cs + 512])
        in_maps.append({
            "xT": xT.astype(bf), "wqk": wqk.astype(bf), "wv": wvs.astype(bf),
            "wo": wos.astype(bf),
            "cosT": cosT, "sinTs": sinTs, "trimask": trimask.astype(bf),
        })
    return in_maps


def _run(in_maps):
    import jax
    r = _get_runner()
    concat = [
        np.concatenate([np.asarray(in_maps[c][n]) for c in range(NCORES)], axis=0)
        for n in r["in_names"]
    ]
    zeros = [
        np.zeros((NCORES * a.shape[0],) + tuple(a.shape[1:]), a.dtype)
        for a in r["out_avals"]
    ]
    outs = r["fn"](*concat, *zeros)
    outs = [np.asarray(o) for o in jax.block_until_ready(outs)]
    per_core = []
    for c in range(NCORES):
        d = {}
        for i, n in enumerate(r["out_names"]):
            shp = r["out_avals"][i].shape
            d[n] = outs[i].reshape((NCORES,) + shp)[c]
        per_core.append(d)
    return per_core


def kernel(x, cos, sin, w_qkv, w_o):
    in_maps = _prep_inputs(x, cos, sin, w_qkv, w_o)
    results = _run(in_maps)
    B = x.shape[0]
    out = np.empty((B, S, D), dtype=np.float32)
    for c in range(NCORES):
        b, tp = c // TPDEG, c % TPDEG
        out[b, :, 512 * tp:512 * tp + 512] = results[c]["out"]
    return out



# revision 13
# speedup vs baseline: 8.9234x; 8.9234x over previous
"""Trainium2 Bass kernel for nn_Attention_84756884619871.

Causal multi-head attention (B=2, S=2048, D=2048, H=16, Dh=128) with RoPE,
fused QKV projection and output projection.

Sharding (8 NeuronCores): data-parallel over batch (2 groups) x
tensor-parallel over heads (4 cores/group, 4 heads each).  Each core:
  - single pass over x^T in 512-column chunks computes q^T,k^T (RoPE
    applied via one ScalarE PSUM evacuation + three bf16 DVE ops) AND v
  - flash-style attention in score-transposed space (p^T[t,s]); softmax
    denominator via ones-vector matmul; no max-subtraction (scores are
    small: exp is safe in fp32)
  - attention runs per 512-column s-quarter, interleaved with the
    projection chunks (quarter q is emitted right after x-chunk q), so
    each of the four AllGathers launches as early as possible and
    overlaps the remaining projection/attention compute
  - the output projections for quarters 0-2 run while the final
    AllGather is in flight; only quarter 3's 64 matmuls trail it
  - DMA queue discipline: weight/x loads + agin stores on the SP HWDGE
    ring, output stores on the ACT ring, ctxg gathers on the Pool SWDGE
    queue, so no long-latency wait ever head-blocks another queue
All matmuls run in bf16 at N=512 (full PE rate, one PSUM bank each).
Host assembles the full [2,2048,2048] output from the 8 column slices.
"""

import numpy as np
import ml_dtypes

import concourse.bass as bass
import concourse.tile as tile
import concourse.mybir as mybir
from concourse import bacc
from concourse.tile import add_dep_helper
from contextlib import ExitStack

F32 = mybir.dt.float32
F32R = mybir.dt.bfloat16  # matmul operand dtype
AF = mybir.ActivationFunctionType

D = 2048
S = 2048
NCORES = 8
TPDEG = 4          # tensor-parallel group size (heads)
HLOC = 4           # heads per core
DH = 128
SCALE = float(1.0 / np.sqrt(DH))
NQ = 4             # s-quarters for attention / collectives
QW = S // NQ       # 512

_STATE: dict = {}
_PHASES: list = []


def _build(reps=1):
    _PHASES.clear()
    nc = bacc.Bacc("TRN2", target_bir_lowering=False, debug=False, num_devices=NCORES)
    xT = nc.dram_tensor("xT", [D, S], F32R, kind="ExternalInput")
    wqk = nc.dram_tensor("wqk", [D, 1024], F32R, kind="ExternalInput")
    wv = nc.dram_tensor("wv", [D, 512], F32R, kind="ExternalInput")
    wo = nc.dram_tensor("wo", [D, 512], F32R, kind="ExternalInput")
    cosT = nc.dram_tensor("cosT", [128, S], F32R, kind="ExternalInput")
    sinTs = nc.dram_tensor("sinTs", [128, S], F32R, kind="ExternalInput")
    trimask = nc.dram_tensor("trimask", [128, 128], F32R, kind="ExternalInput")
    out = nc.dram_tensor("out", [S, 512], F32, kind="ExternalOutput")

    xT3 = xT.ap().rearrange("(ko ki) s -> ki ko s", ki=128)
    wqk3 = wqk.ap().rearrange("(ko ki) c -> ki ko c", ki=128)
    wv3 = wv.ap().rearrange("(ko ki) c -> ki ko c", ki=128)
    wo3 = wo.ap().rearrange("(ko ki) c -> ki ko c", ki=128)

    with tile.TileContext(nc) as tc, ExitStack() as top:
        # ---- persistent constants -----------------------------------------
        per = top.enter_context(tc.tile_pool(name="persist", bufs=1))
        mask_sb = per.tile([128, 128], F32R, name="mask")
        nc.sync.dma_start(mask_sb[:], trimask.ap())
        ones_c0 = per.tile([128, 128], F32, name="ones_c0")
        nc.vector.memset(ones_c0[:], 1.0)
        ones_sq = per.tile([128, 128], F32R, name="ones_sq")
        nc.vector.tensor_copy(ones_sq[:], ones_c0[:])

        dram = top.enter_context(tc.tile_pool(name="dram", bufs=1, space="DRAM"))
        agin = [dram.tile([HLOC * 128, QW], F32R, name=f"agin{q}") for q in range(NQ)]
        agout = [dram.tile([D, QW], F32R, name=f"agout{q}") for q in range(NQ)]

        qkv = top.enter_context(ExitStack())
        qk_pool = qkv.enter_context(tc.tile_pool(name="qkpool", bufs=1))
        qrot = [qk_pool.tile([128, S], F32R, name=f"qrot{h}") for h in range(HLOC)]
        krot = [qk_pool.tile([128, S], F32R, name=f"krot{h}") for h in range(HLOC)]
        v_pool = qkv.enter_context(tc.tile_pool(name="vpool", bufs=1))
        vsb = [v_pool.tile([128, 512], F32R, name=f"v{j}") for j in range(16)]

        wo_pool = top.enter_context(tc.tile_pool(name="wop", bufs=1))
        wo_sb = wo_pool.tile([128, 16, 512], F32R, name="wo_sb")

        for _rep in range(reps):
            _emit_rep(nc, tc, _rep, xT3, wqk3, wv3, wo3, cosT, sinTs, out,
                      mask_sb, ones_sq, agin, agout,
                      qrot, krot, vsb, wo_sb)

    _PHASES.append(('end', nc.next_id()))
    nc.compile()
    return nc


def _emit_rep(nc, tc, rep, xT3, wqk3, wv3, wo3, cosT, sinTs, out,
              mask_sb, ones_sq, agin, agout,
              qrot, krot, vsb, wo_sb):
    """One full kernel body: A-chunks (projection+RoPE+v) interleaved with
    attention quarters so each AllGather launches as early as possible;
    quarter 3 is woven with the earlier quarters' output projections."""
    last_pe = [None]

    with ExitStack() as outer:
        p_pool = outer.enter_context(tc.tile_pool(name="pp", bufs=4))
        misc = outer.enter_context(tc.tile_pool(name="miscb", bufs=2))

        def attn_quarter(sq, sc_ps, ctx_ps, l_ps, after_head=None):
            _PHASES.append((f'attn{sq}', nc.next_id()))
            for h in range(HLOC):
                ctx = ctx_ps.tile([128, QW], F32, tag="ctx", name=f"ctx{sq}_{h}")
                lps = l_ps.tile([128, QW], F32, tag="l", name=f"l{sq}_{h}")
                jmax = 4 * sq + 4
                for j in range(jmax):
                    dj = j - 4 * sq
                    c0 = max(0, 128 * dj)
                    sct = sc_ps.tile([128, QW], F32, tag="scps",
                                     name=f"sc{sq}_{h}_{j}")
                    nc.tensor.matmul(
                        sct[:, c0:QW], krot[h][:, 128 * j:128 * j + 128],
                        qrot[h][:, QW * sq + c0:QW * sq + QW],
                        start=True, stop=True,
                    )
                    p_t = p_pool.tile([128, QW], F32R, tag="p",
                                      name=f"p{sq}_{h}_{j}")
                    nc.scalar.activation(p_t[:, c0:QW], sct[:, c0:QW],
                                         AF.Exp, scale=SCALE)
                    if dj >= 0:
                        dsl = slice(c0, c0 + 128)
                        nc.vector.tensor_mul(p_t[:, dsl], p_t[:, dsl], mask_sb[:])
                    last = (j == jmax - 1)
                    pv_mm = nc.tensor.matmul(
                        ctx[:, c0:QW], vsb[j][:, 128 * h:128 * h + 128],
                        p_t[:, c0:QW], start=(j == 0), stop=last,
                        skip_group_check=True,
                    )
                    last_pe[0] = pv_mm
                    nc.tensor.matmul(
                        lps[:, c0:QW], ones_sq[:], p_t[:, c0:QW],
                        start=(j == 0), stop=last, skip_group_check=True,
                    )
                # normalize: ctxn = ctx * (1/l); the ones-128 stationary
                # already broadcast l across all partitions, so a plain
                # elementwise reciprocal+mul suffices (no PE broadcast)
                linv = misc.tile([128, QW], F32, tag="linv", name=f"li{sq}_{h}")
                nc.vector.reciprocal_approx_fast(out=linv[:], in_=lps[:])
                ctxn = misc.tile([128, QW], F32R, tag="ctxn", name=f"cn{sq}_{h}")
                nc.vector.tensor_mul(ctxn[:], ctx[:], linv[:])
                # SP-ring HWDGE: every A-phase load is emitted before any
                # attention, so the SP queue is drained by now; this keeps
                # the ctxn cross-engine wait off the ACT sequencer, which is
                # the bottleneck engine inside quarter 3
                nc.sync.dma_start(agin[sq][128 * h:128 * h + 128, :], ctxn[:])
                if after_head is not None:
                    after_head(h)
            _PHASES.append((f'AG{sq}', nc.next_id()))
            nc.gpsimd.collective_compute(
                "AllGather", mybir.AluOpType.bypass,
                ins=[agin[sq][:]], outs=[agout[sq][:]],
                replica_groups=[[0, 1, 2, 3], [4, 5, 6, 7]],
            )

        with ExitStack() as st:
            wqk_pool = st.enter_context(tc.tile_pool(name="wqkp", bufs=1))
            wqk_sb = wqk_pool.tile([128, 16, 1024], F32R, name="wqk_sb")
            wv_pool = st.enter_context(tc.tile_pool(name="wvp", bufs=1))
            wv_sb = wv_pool.tile([128, 16, 512], F32R, name="wv_sb")
            cs_pool = st.enter_context(tc.tile_pool(name="csp", bufs=1))
            cos_sb = cs_pool.tile([128, S], F32R, name="cos_sb")
            sin_sb = cs_pool.tile([128, S], F32R, name="sin_sb")
            xt_pool = st.enter_context(tc.tile_pool(name="xtp", bufs=3))
            xt_tiles = [None] * NQ

            def xt_dma(sc, part=None):
                if xt_tiles[sc] is None:
                    xt_tiles[sc] = xt_pool.tile([128, 16, QW], F32R, tag="xt",
                                                name=f"xt{sc}")
                t = xt_tiles[sc]
                ssl = slice(QW * sc, QW * sc + QW)
                if part is None:
                    nc.sync.dma_start(t[:], xT3[:, :, ssl])
                else:
                    nc.sync.dma_start(t[:, 4 * part:4 * part + 4, :],
                                      xT3[:, 4 * part:4 * part + 4, ssl])

            # startup-critical DMA order: first weight group + first x slab
            # land before the rest so the first matmuls can issue early.
            # wv/cos/sin go on the ACT ring (idle until the c-phase stores)
            # so the SP ring only carries the wqk+x bytes the first q/k
            # matmul chains are actually waiting on.
            nc.sync.dma_start(wqk_sb[:, 0:4, :], wqk3[:, 0:4, :])
            xt_dma(0, 0)
            for g in range(1, 4):
                nc.sync.dma_start(wqk_sb[:, 4 * g:4 * g + 4, :],
                                  wqk3[:, 4 * g:4 * g + 4, :])
                xt_dma(0, g)
            for g in range(4):
                nc.scalar.dma_start(wv_sb[:, 4 * g:4 * g + 4, :],
                                    wv3[:, 4 * g:4 * g + 4, :])
            nc.scalar.dma_start(cos_sb[:], cosT.ap())
            nc.scalar.dma_start(sin_sb[:], sinTs.ap())
            xt_dma(1)
            xt_dma(2)
            if rep == 0:
                for g in range(4):
                    nc.scalar.dma_start(wo_sb[:, 4 * g:4 * g + 4, :],
                                        wo3[:, 4 * g:4 * g + 4, :])

            rope_pool = st.enter_context(tc.tile_pool(name="ropep", bufs=2))
            ps_qk = st.enter_context(tc.tile_pool(name="psqk", bufs=2,
                                                  space="PSUM"))
            ps_v = st.enter_context(tc.tile_pool(name="psv", bufs=2,
                                                 space="PSUM"))
            sc_e = st.enter_context(tc.tile_pool(name="scpse", bufs=2,
                                                 space="PSUM"))
            ctx_e = st.enter_context(tc.tile_pool(name="ctxpse", bufs=1,
                                                  space="PSUM"))
            l_e = st.enter_context(tc.tile_pool(name="lpse", bufs=1,
                                                space="PSUM"))

            def a_chunk(sc):
                _PHASES.append((f'A:sc{sc}', nc.next_id()))
                sl = slice(QW * sc, QW * sc + QW)
                xt_c = xt_tiles[sc]
                for m in range(8):
                    pq = ps_qk.tile([128, QW], F32, tag="pq", name=f"pq{sc}_{m}")
                    for ko in range(16):
                        nc.tensor.matmul(pq[:],
                                         wqk_sb[:, ko, 128 * m:128 * m + 128],
                                         xt_c[:, ko, :],
                                         start=(ko == 0), stop=(ko == 15))
                    pqb = rope_pool.tile([128, QW], F32R, tag="pqb",
                                         name=f"pqb{sc}_{m}")
                    nc.scalar.copy(pqb[:], pq[:])
                    t1 = rope_pool.tile([128, QW], F32R, tag="t1",
                                        name=f"t1_{sc}_{m}")
                    nc.vector.tensor_mul(t1[:], pqb[:], cos_sb[:, sl])
                    t2 = rope_pool.tile([128, QW], F32R, tag="t2",
                                        name=f"t2_{sc}_{m}")
                    # rotate-half reads must come from PSUM: a partition-base
                    # mismatch between two SBUF operands is rejected by the
                    # BIR verifier, but PSUM+SBUF operands carry independent
                    # base-partition offsets
                    nc.vector.tensor_mul(t2[0:64, :], pq[64:128, :],
                                         sin_sb[0:64, sl])
                    nc.vector.tensor_mul(t2[64:128, :], pq[0:64, :],
                                         sin_sb[64:128, sl])
                    dest = (qrot[m] if m < 4 else krot[m - 4])[:, sl]
                    nc.vector.tensor_add(dest, t1[:], t2[:])
                a_chunk_v(sc)

            def a_chunk_v(sc):
                xt_c = xt_tiles[sc]
                for u in range(4):
                    j = 4 * sc + u
                    pv = ps_v.tile([128, 512], F32, tag="pv", name=f"pv{j}")
                    for ko in range(16):
                        nc.tensor.matmul(
                            pv[:], xt_c[:, ko, 128 * u:128 * u + 128],
                            wv_sb[:, ko, :], start=(ko == 0), stop=(ko == 15),
                        )
                    nc.scalar.copy(vsb[j][:], pv[:])

            a_chunk(0)
            xt_dma(3)
            attn_quarter(0, sc_e, ctx_e, l_e)
            a_chunk(1)
            attn_quarter(1, sc_e, ctx_e, l_e)
            a_chunk(2)
            attn_quarter(2, sc_e, ctx_e, l_e)
            a_chunk(3)

        # ---- quarter 3 + phase C woven (A pools released) -----------------
        with ExitStack() as st:
            cg_pool = st.enter_context(tc.tile_pool(name="cgp", bufs=4))
            osb_pool = st.enter_context(tc.tile_pool(name="osbp", bufs=2))
            ps_o = st.enter_context(tc.tile_pool(name="pso", bufs=2,
                                                 space="PSUM"))
            sc_l = st.enter_context(tc.tile_pool(name="scpsl", bufs=3,
                                                 space="PSUM"))
            ctx_l = st.enter_context(tc.tile_pool(name="ctxpsl", bufs=2,
                                                  space="PSUM"))
            l_l = st.enter_context(tc.tile_pool(name="lpsl", bufs=1,
                                                space="PSUM"))
            ctxg = [None] * NQ

            def cg_load(sq):
                _PHASES.append((f'cg{sq}', nc.next_id()))
                cg = cg_pool.tile([128, 16, QW], F32R, tag="cg",
                                  name=f"ctxg{sq}")
                ag3 = agout[sq].rearrange("(ko ki) s -> ki ko s", ki=128)
                # SWDGE queue: must not head-block the HWDGE rings.
                # Split per ko-group so the c-block matmuls for group g
                # only wait on their own slice (pipelines the gather load
                # with the output projection instead of serializing).
                for g4 in range(4):
                    nc.gpsimd.dma_start(cg[:, 4 * g4:4 * g4 + 4, :],
                                        ag3[:, 4 * g4:4 * g4 + 4, :])
                ctxg[sq] = cg

            def c_block(sq, mms):
                _PHASES.append((f'c{sq}.{mms[0]}', nc.next_id()))
                anchor = last_pe[0]
                for mm in mms:
                    m = 4 * sq + mm
                    po = ps_o.tile([128, 512], F32, tag="pso", name=f"po{m}")
                    for ko in range(16):
                        o_mm = nc.tensor.matmul(
                            po[:], ctxg[sq][:, ko, 128 * mm:128 * mm + 128],
                            wo_sb[:, ko, :], start=(ko == 0), stop=(ko == 15),
                        )
                        if ko == 0 and anchor is not None:
                            # keep the scheduler from hoisting the output
                            # projection ahead of attention compute that
                            # does not depend on the AllGather
                            add_dep_helper(o_mm.ins, anchor.ins, sync=True,
                                           reason="order C after attention")
                        anchor = o_mm
                        last_pe[0] = o_mm
                    osb = osb_pool.tile([128, 512], F32, tag="osb",
                                        name=f"osb{m}")
                    nc.scalar.copy(osb[:], po[:])
                    # ACT ring: producer (scalar.copy) is on the same engine,
                    # so the store issues with no cross-engine wait and stays
                    # clear of the Pool queue that carries the cg loads
                    nc.scalar.dma_start(out.ap()[128 * m:128 * m + 128, :],
                                        osb[:])

            # cg3 is emitted late: its wait on the final AllGather must not
            # head-block the Pool queue in front of earlier quarters' traffic
            cg_load(0)
            cg_load(1)
            cg_load(2)

            attn_quarter(3, sc_l, ctx_l, l_l)
            c_block(0, [0, 1, 2, 3])
            c_block(1, [0, 1, 2, 3])
            c_block(2, [0, 1, 2, 3])
            cg_load(3)
            c_block(3, [0, 1, 2, 3])


def _get_runner():
    """Build (once) a persistent jitted SPMD executor for the kernel program."""
    if "runner" in _STATE:
        return _STATE["runner"]
    import jax
    from jax.sharding import Mesh, PartitionSpec
    from jax.experimental.shard_map import shard_map
    from concourse import bass2jax

    nc = _build()
    bass2jax.install_neuronx_cc_hook()

    in_names, out_names, out_avals = [], [], []
    for alloc in nc.m.functions[0].allocations:
        if not isinstance(alloc, mybir.MemoryLocationSet):
            continue
        name = alloc.memorylocations[0].name
        pname = nc.partition_id_tensor.name if nc.partition_id_tensor else None
        if alloc.kind == "ExternalInput":
            if name != pname:
                in_names.append(name)
        elif alloc.kind == "ExternalOutput":
            out_names.append(name)
            out_avals.append(
                jax.core.ShapedArray(tuple(alloc.tensor_shape),
                                     mybir.dt.np(alloc.dtype))
            )
    n_params = len(in_names)
    all_in = list(in_names) + list(out_names)
    pname = nc.partition_id_tensor.name if nc.partition_id_tensor else None
    if pname is not None:
        all_in.append(pname)

    def _body(*args):
        operands = list(args)
        if pname is not None:
            operands.append(bass2jax.partition_id_tensor())
        outs = bass2jax._bass_exec_p.bind(
            *operands,
            out_avals=tuple(out_avals),
            in_names=tuple(all_in),
            out_names=tuple(out_names),
            lowering_input_output_aliases=(),
            sim_require_finite=False,
            sim_require_nnan=False,
            nc=nc,
        )
        return tuple(outs)

    devices = jax.devices()[:NCORES]
    mesh = Mesh(np.asarray(devices), ("core",))
    specs = (PartitionSpec("core"),)
    sharded = jax.jit(
        shard_map(
            _body, mesh=mesh,
            in_specs=specs * (n_params + len(out_names)),
            out_specs=specs * len(out_names),
            check_rep=False,
        ),
        keep_unused=True,
    )
    runner = {
        "fn": sharded, "in_names": in_names, "out_names": out_names,
        "out_avals": out_avals, "n_params": n_params,
    }
    _STATE["runner"] = runner
    return runner


def _prep_inputs(x, cos, sin, w_qkv, w_o):
    """Host-side sharding: per-core input dict list."""
    x = np.asarray(x, dtype=np.float32)
    cos = np.asarray(cos, dtype=np.float32)
    sin = np.asarray(sin, dtype=np.float32)
    w_qkv = np.asarray(w_qkv, dtype=np.float32)
    w_o = np.asarray(w_o, dtype=np.float32)

    bf = ml_dtypes.bfloat16
    cosT = np.ascontiguousarray(cos.T).astype(bf)           # [128, S]
    sinT = sin.T
    sinTs = np.ascontiguousarray(
        np.concatenate([-sinT[0:64], sinT[64:128]], axis=0)).astype(bf)
    pp, ff = np.meshgrid(np.arange(128), np.arange(128), indexing="ij")
    trimask = (pp <= ff).astype(np.float32)                 # t <= s

    in_maps = []
    for c in range(NCORES):
        b, tp = c // TPDEG, c % TPDEG
        cs = 512 * tp
        xT = np.ascontiguousarray(x[b].T)                   # [D, S]
        wq = w_qkv[:, cs:cs + 512]
        wk = w_qkv[:, D + cs:D + cs + 512]
        wqk = np.ascontiguousarray(np.concatenate([wq, wk], axis=1))
        wvs = np.ascontiguousarray(w_qkv[:, 2 * D + cs:2 * D + cs + 512])
        wos = np.ascontiguousarray(w_o[:, cs:cs + 512])
        in_maps.append({
            "xT": xT.astype(bf), "wqk": wqk.astype(bf), "wv": wvs.astype(bf),
            "wo": wos.astype(bf),
            "cosT": cosT, "sinTs": sinTs, "trimask": trimask.astype(bf),
        })
    return in_maps


def _run(in_maps):
    import jax
    r = _get_runner()
    concat = [
        np.concatenate([np.asarray(in_maps[c][n]) for c in range(NCORES)], axis=0)
        for n in r["in_names"]
    ]
    zeros = [
        np.zeros((NCORES * a.shape[0],) + tuple(a.shape[1:]), a.dtype)
        for a in r["out_avals"]
    ]
    outs = r["fn"](*concat, *zeros)
    outs = [np.asarray(o) for o in jax.block_until_ready(outs)]
    per_core = []
    for c in range(NCORES):
        d = {}
        for i, n in enumerate(r["out_names"]):
            shp = r["out_avals"][i].shape
            d[n] = outs[i].reshape((NCORES,) + shp)[c]
        per_core.append(d)
    return per_core


def kernel(x, cos, sin, w_qkv, w_o):
    in_maps = _prep_inputs(x, cos, sin, w_qkv, w_o)
    results = _run(in_maps)
    B = x.shape[0]
    out = np.empty((B, S, D), dtype=np.float32)
    for c in range(NCORES):
        b, tp = c // TPDEG, c % TPDEG
        out[b, :, 512 * tp:512 * tp + 512] = results[c]["out"]
    return out



# revision 16
# speedup vs baseline: 9.0881x; 1.0185x over previous
"""Trainium2 Bass kernel for nn_Attention_84756884619871.

Causal multi-head attention (B=2, S=2048, D=2048, H=16, Dh=128) with RoPE,
fused QKV projection and output projection.

Sharding (8 NeuronCores): data-parallel over batch (2 groups) x
tensor-parallel over heads (4 cores/group, 4 heads each).  Each core:
  - single pass over x^T in 512-column chunks computes q^T,k^T (RoPE
    applied via one ScalarE PSUM evacuation + three bf16 DVE ops) AND v
  - flash-style attention in score-transposed space (p^T[t,s]); softmax
    denominator via ones-vector matmul; no max-subtraction (scores are
    small: exp is safe in fp32)
  - attention runs per 512-column s-quarter, interleaved with the
    projection chunks (quarter q is emitted right after x-chunk q), so
    each of the four AllGathers launches as early as possible and
    overlaps the remaining projection/attention compute
  - the output projections for quarters 0-2 run while the final
    AllGather is in flight; only quarter 3's 64 matmuls trail it
  - DMA queue discipline: weight/x loads + agin stores on the SP HWDGE
    ring, output stores on the ACT ring, ctxg gathers on the Pool SWDGE
    queue, so no long-latency wait ever head-blocks another queue
All matmuls run in bf16 at N=512 (full PE rate, one PSUM bank each).
Host assembles the full [2,2048,2048] output from the 8 column slices.
"""

import numpy as np
import ml_dtypes

import concourse.bass as bass
import concourse.tile as tile
import concourse.mybir as mybir
from concourse import bacc
from concourse.tile import add_dep_helper
from contextlib import ExitStack

F32 = mybir.dt.float32
F32R = mybir.dt.bfloat16  # matmul operand dtype
AF = mybir.ActivationFunctionType

D = 2048
S = 2048
NCORES = 8
TPDEG = 4          # tensor-parallel group size (heads)
HLOC = 4           # heads per core
DH = 128
SCALE = float(1.0 / np.sqrt(DH))
NQ = 4             # s-quarters for attention / collectives
QW = S // NQ       # 512

_STATE: dict = {}
_PHASES: list = []


def _build(reps=1):
    _PHASES.clear()
    nc = bacc.Bacc("TRN2", target_bir_lowering=False, debug=False, num_devices=NCORES)
    xT = nc.dram_tensor("xT", [D, S], F32R, kind="ExternalInput")
    wqk = nc.dram_tensor("wqk", [D, 1024], F32R, kind="ExternalInput")
    wv = nc.dram_tensor("wv", [D, 512], F32R, kind="ExternalInput")
    wo = nc.dram_tensor("wo", [D, 512], F32R, kind="ExternalInput")
    cosT = nc.dram_tensor("cosT", [128, S], F32R, kind="ExternalInput")
    sinTs = nc.dram_tensor("sinTs", [128, S], F32R, kind="ExternalInput")
    trimask = nc.dram_tensor("trimask", [128, 128], F32R, kind="ExternalInput")
    out = nc.dram_tensor("out", [S, 512], F32, kind="ExternalOutput")

    xT3 = xT.ap().rearrange("(ko ki) s -> ki ko s", ki=128)
    wqk3 = wqk.ap().rearrange("(ko ki) c -> ki ko c", ki=128)
    wv3 = wv.ap().rearrange("(ko ki) c -> ki ko c", ki=128)
    wo3 = wo.ap().rearrange("(ko ki) c -> ki ko c", ki=128)

    with tile.TileContext(nc) as tc, ExitStack() as top:
        # ---- persistent constants -----------------------------------------
        per = top.enter_context(tc.tile_pool(name="persist", bufs=1))
        mask_sb = per.tile([128, 128], F32R, name="mask")
        nc.sync.dma_start(mask_sb[:], trimask.ap())
        ones_c0 = per.tile([128, 128], F32, name="ones_c0")
        nc.vector.memset(ones_c0[:], 1.0)
        ones_sq = per.tile([128, 128], F32R, name="ones_sq")
        nc.vector.tensor_copy(ones_sq[:], ones_c0[:])

        dram = top.enter_context(tc.tile_pool(name="dram", bufs=1, space="DRAM"))
        agin = [dram.tile([HLOC * 128, QW], F32R, name=f"agin{q}") for q in range(NQ)]
        agout = [dram.tile([D, QW], F32R, name=f"agout{q}") for q in range(NQ)]

        qkv = top.enter_context(ExitStack())
        qk_pool = qkv.enter_context(tc.tile_pool(name="qkpool", bufs=1))
        qrot = [qk_pool.tile([128, S], F32R, name=f"qrot{h}") for h in range(HLOC)]
        krot = [qk_pool.tile([128, S], F32R, name=f"krot{h}") for h in range(HLOC)]
        v_pool = qkv.enter_context(tc.tile_pool(name="vpool", bufs=1))
        vsb = [v_pool.tile([128, 512], F32R, name=f"v{j}") for j in range(16)]

        wo_pool = top.enter_context(tc.tile_pool(name="wop", bufs=1))
        wo_sb = wo_pool.tile([128, 16, 512], F32R, name="wo_sb")

        for _rep in range(reps):
            _emit_rep(nc, tc, _rep, xT3, wqk3, wv3, wo3, cosT, sinTs, out,
                      mask_sb, ones_sq, agin, agout,
                      qrot, krot, vsb, wo_sb)

    _PHASES.append(('end', nc.next_id()))
    nc.compile()
    return nc


def _emit_rep(nc, tc, rep, xT3, wqk3, wv3, wo3, cosT, sinTs, out,
              mask_sb, ones_sq, agin, agout,
              qrot, krot, vsb, wo_sb):
    """One full kernel body: A-chunks (projection+RoPE+v) interleaved with
    attention quarters so each AllGather launches as early as possible;
    quarter 3 is woven with the earlier quarters' output projections."""
    last_pe = [None]

    with ExitStack() as outer:
        p_pool = outer.enter_context(tc.tile_pool(name="pp", bufs=4))
        misc = outer.enter_context(tc.tile_pool(name="miscb", bufs=2))

        def attn_quarter(sq, sc_ps, ctx_ps, l_ps, after_head=None):
            _PHASES.append((f'attn{sq}', nc.next_id()))
            for h in range(HLOC):
                ctx = ctx_ps.tile([128, QW], F32, tag="ctx", name=f"ctx{sq}_{h}")
                lps = l_ps.tile([128, QW], F32, tag="l", name=f"l{sq}_{h}")
                jmax = 4 * sq + 4
                for j in range(jmax):
                    dj = j - 4 * sq
                    c0 = max(0, 128 * dj)
                    sct = sc_ps.tile([128, QW], F32, tag="scps",
                                     name=f"sc{sq}_{h}_{j}")
                    nc.tensor.matmul(
                        sct[:, c0:QW], krot[h][:, 128 * j:128 * j + 128],
                        qrot[h][:, QW * sq + c0:QW * sq + QW],
                        start=True, stop=True,
                    )
                    p_t = p_pool.tile([128, QW], F32R, tag="p",
                                      name=f"p{sq}_{h}_{j}")
                    nc.scalar.activation(p_t[:, c0:QW], sct[:, c0:QW],
                                         AF.Exp, scale=SCALE)
                    if dj >= 0:
                        dsl = slice(c0, c0 + 128)
                        nc.vector.tensor_mul(p_t[:, dsl], p_t[:, dsl], mask_sb[:])
                    last = (j == jmax - 1)
                    pv_mm = nc.tensor.matmul(
                        ctx[:, c0:QW], vsb[j][:, 128 * h:128 * h + 128],
                        p_t[:, c0:QW], start=(j == 0), stop=last,
                        skip_group_check=True,
                    )
                    last_pe[0] = pv_mm
                    nc.tensor.matmul(
                        lps[:, c0:QW], ones_sq[:], p_t[:, c0:QW],
                        start=(j == 0), stop=last, skip_group_check=True,
                    )
                # normalize: ctxn = ctx * (1/l); the ones-128 stationary
                # already broadcast l across all partitions, so a plain
                # elementwise reciprocal+mul suffices (no PE broadcast)
                linv = misc.tile([128, QW], F32, tag="linv", name=f"li{sq}_{h}")
                nc.vector.reciprocal_approx_fast(out=linv[:], in_=lps[:])
                ctxn = misc.tile([128, QW], F32R, tag="ctxn", name=f"cn{sq}_{h}")
                nc.vector.tensor_mul(ctxn[:], ctx[:], linv[:])
                # SP-ring HWDGE: every A-phase load is emitted before any
                # attention, so the SP queue is drained by now; this keeps
                # the ctxn cross-engine wait off the ACT sequencer, which is
                # the bottleneck engine inside quarter 3
                nc.sync.dma_start(agin[sq][128 * h:128 * h + 128, :], ctxn[:])
                if after_head is not None:
                    after_head(h)
            _PHASES.append((f'AG{sq}', nc.next_id()))
            nc.gpsimd.collective_compute(
                "AllGather", mybir.AluOpType.bypass,
                ins=[agin[sq][:]], outs=[agout[sq][:]],
                replica_groups=[[0, 1, 2, 3], [4, 5, 6, 7]],
            )

        with ExitStack() as st:
            wqk_pool = st.enter_context(tc.tile_pool(name="wqkp", bufs=1))
            wqk_sb = wqk_pool.tile([128, 16, 1024], F32R, name="wqk_sb")
            wv_pool = st.enter_context(tc.tile_pool(name="wvp", bufs=1))
            wv_sb = wv_pool.tile([128, 16, 512], F32R, name="wv_sb")
            cs_pool = st.enter_context(tc.tile_pool(name="csp", bufs=1))
            cos_sb = cs_pool.tile([128, S], F32R, name="cos_sb")
            sin_sb = cs_pool.tile([128, S], F32R, name="sin_sb")
            xt_pool = st.enter_context(tc.tile_pool(name="xtp", bufs=3))
            xt_tiles = [None] * NQ

            def xt_dma(sc, part=None):
                if xt_tiles[sc] is None:
                    xt_tiles[sc] = xt_pool.tile([128, 16, QW], F32R, tag="xt",
                                                name=f"xt{sc}")
                t = xt_tiles[sc]
                ssl = slice(QW * sc, QW * sc + QW)
                if part is None:
                    nc.sync.dma_start(t[:], xT3[:, :, ssl])
                else:
                    nc.sync.dma_start(t[:, 4 * part:4 * part + 4, :],
                                      xT3[:, 4 * part:4 * part + 4, ssl])

            # startup-critical DMA order: first weight group + first x slab
            # land before the rest so the first matmuls can issue early.
            # wv/cos/sin go on the ACT ring (idle until the c-phase stores)
            # so the SP ring only carries the wqk+x bytes the first q/k
            # matmul chains are actually waiting on.
            nc.sync.dma_start(wqk_sb[:, 0:4, :], wqk3[:, 0:4, :])
            for g in range(4):
                xt_dma(0, g)
            for g in range(1, 4):
                nc.sync.dma_start(wqk_sb[:, 4 * g:4 * g + 4, :],
                                  wqk3[:, 4 * g:4 * g + 4, :])
            for g in range(4):
                nc.scalar.dma_start(wv_sb[:, 4 * g:4 * g + 4, :],
                                    wv3[:, 4 * g:4 * g + 4, :])
            nc.scalar.dma_start(cos_sb[:], cosT.ap())
            nc.scalar.dma_start(sin_sb[:], sinTs.ap())
            xt_dma(1)
            xt_dma(2)
            if rep == 0:
                for g in range(4):
                    nc.scalar.dma_start(wo_sb[:, 4 * g:4 * g + 4, :],
                                        wo3[:, 4 * g:4 * g + 4, :])

            rope_pool = st.enter_context(tc.tile_pool(name="ropep", bufs=2))
            ps_qk = st.enter_context(tc.tile_pool(name="psqk", bufs=2,
                                                  space="PSUM"))
            ps_v = st.enter_context(tc.tile_pool(name="psv", bufs=2,
                                                 space="PSUM"))
            sc_e = st.enter_context(tc.tile_pool(name="scpse", bufs=2,
                                                 space="PSUM"))
            ctx_e = st.enter_context(tc.tile_pool(name="ctxpse", bufs=1,
                                                  space="PSUM"))
            l_e = st.enter_context(tc.tile_pool(name="lpse", bufs=1,
                                                space="PSUM"))

            def a_chunk(sc):
                _PHASES.append((f'A:sc{sc}', nc.next_id()))
                sl = slice(QW * sc, QW * sc + QW)
                xt_c = xt_tiles[sc]
                # chunk 0 is DMA-paced (the 4MB wqk stream outruns the SP
                # ring): weave the v chains — fed by the parallel ACT-ring
                # wv load — between the first q/k chains so the PE has
                # work while the next wqk group lands
                morder = ([('m', 0), ('v', 0), ('m', 1), ('v', 1), ('m', 2),
                           ('v', 2), ('m', 3), ('v', 3), ('m', 4), ('m', 5),
                           ('m', 6), ('m', 7)] if sc == 0 else
                          [('m', m) for m in range(8)] +
                          [('v', u) for u in range(4)])
                for kind, idx in morder:
                    if kind == 'v':
                        a_chunk_v1(sc, idx)
                        continue
                    m = idx
                    pq = ps_qk.tile([128, QW], F32, tag="pq", name=f"pq{sc}_{m}")
                    for ko in range(16):
                        nc.tensor.matmul(pq[:],
                                         wqk_sb[:, ko, 128 * m:128 * m + 128],
                                         xt_c[:, ko, :],
                                         start=(ko == 0), stop=(ko == 15))
                    pqb = rope_pool.tile([128, QW], F32R, tag="pqb",
                                         name=f"pqb{sc}_{m}")
                    nc.scalar.copy(pqb[:], pq[:])
                    t1 = rope_pool.tile([128, QW], F32R, tag="t1",
                                        name=f"t1_{sc}_{m}")
                    nc.vector.tensor_mul(t1[:], pqb[:], cos_sb[:, sl])
                    t2 = rope_pool.tile([128, QW], F32R, tag="t2",
                                        name=f"t2_{sc}_{m}")
                    # rotate-half reads must come from PSUM: a partition-base
                    # mismatch between two SBUF operands is rejected by the
                    # BIR verifier, but PSUM+SBUF operands carry independent
                    # base-partition offsets
                    nc.vector.tensor_mul(t2[0:64, :], pq[64:128, :],
                                         sin_sb[0:64, sl])
                    nc.vector.tensor_mul(t2[64:128, :], pq[0:64, :],
                                         sin_sb[64:128, sl])
                    dest = (qrot[m] if m < 4 else krot[m - 4])[:, sl]
                    nc.vector.tensor_add(dest, t1[:], t2[:])

            def a_chunk_v1(sc, u):
                xt_c = xt_tiles[sc]
                j = 4 * sc + u
                pv = ps_v.tile([128, 512], F32, tag="pv", name=f"pv{j}")
                for ko in range(16):
                    nc.tensor.matmul(
                        pv[:], xt_c[:, ko, 128 * u:128 * u + 128],
                        wv_sb[:, ko, :], start=(ko == 0), stop=(ko == 15),
                    )
                nc.scalar.copy(vsb[j][:], pv[:])

            a_chunk(0)
            xt_dma(3)
            attn_quarter(0, sc_e, ctx_e, l_e)
            a_chunk(1)
            attn_quarter(1, sc_e, ctx_e, l_e)
            a_chunk(2)
            attn_quarter(2, sc_e, ctx_e, l_e)
            a_chunk(3)

        # ---- quarter 3 + phase C woven (A pools released) -----------------
        with ExitStack() as st:
            cg_pool = st.enter_context(tc.tile_pool(name="cgp", bufs=4))
            osb_pool = st.enter_context(tc.tile_pool(name="osbp", bufs=2))
            ps_o = st.enter_context(tc.tile_pool(name="pso", bufs=2,
                                                 space="PSUM"))
            sc_l = st.enter_context(tc.tile_pool(name="scpsl", bufs=3,
                                                 space="PSUM"))
            ctx_l = st.enter_context(tc.tile_pool(name="ctxpsl", bufs=2,
                                                  space="PSUM"))
            l_l = st.enter_context(tc.tile_pool(name="lpsl", bufs=1,
                                                space="PSUM"))
            ctxg = [None] * NQ

            def cg_load(sq):
                _PHASES.append((f'cg{sq}', nc.next_id()))
                cg = cg_pool.tile([128, 16, QW], F32R, tag="cg",
                                  name=f"ctxg{sq}")
                ag3 = agout[sq].rearrange("(ko ki) s -> ki ko s", ki=128)
                # SWDGE queue: must not head-block the HWDGE rings.
                # Split per ko-group so the c-block matmuls for group g
                # only wait on their own slice (pipelines the gather load
                # with the output projection instead of serializing).
                for g4 in range(4):
                    nc.gpsimd.dma_start(cg[:, 4 * g4:4 * g4 + 4, :],
                                        ag3[:, 4 * g4:4 * g4 + 4, :])
                ctxg[sq] = cg

            def c_block(sq, mms):
                _PHASES.append((f'c{sq}.{mms[0]}', nc.next_id()))
                anchor = last_pe[0]
                for mm in mms:
                    m = 4 * sq + mm
                    po = ps_o.tile([128, 512], F32, tag="pso", name=f"po{m}")
                    for ko in range(16):
                        o_mm = nc.tensor.matmul(
                            po[:], ctxg[sq][:, ko, 128 * mm:128 * mm + 128],
                            wo_sb[:, ko, :], start=(ko == 0), stop=(ko == 15),
                        )
                        if ko == 0 and anchor is not None:
                            # keep the scheduler from hoisting the output
                            # projection ahead of attention compute that
                            # does not depend on the AllGather
                            add_dep_helper(o_mm.ins, anchor.ins, sync=True,
                                           reason="order C after attention")
                        anchor = o_mm
                        last_pe[0] = o_mm
                    osb = osb_pool.tile([128, 512], F32, tag="osb",
                                        name=f"osb{m}")
                    nc.scalar.copy(osb[:], po[:])
                    # ACT ring: producer (scalar.copy) is on the same engine,
                    # so the store issues with no cross-engine wait and stays
                    # clear of the Pool queue that carries the cg loads
                    nc.scalar.dma_start(out.ap()[128 * m:128 * m + 128, :],
                                        osb[:])

            # cg3 is emitted late: its wait on the final AllGather must not
            # head-block the Pool queue in front of earlier quarters' traffic
            cg_load(0)
            cg_load(1)
            cg_load(2)

            attn_quarter(3, sc_l, ctx_l, l_l)
            c_block(0, [0, 1, 2, 3])
            c_block(1, [0, 1, 2, 3])
            c_block(2, [0, 1, 2, 3])
            cg_load(3)
            c_block(3, [0, 1, 2, 3])


def _get_runner():
    """Build (once) a persistent jitted SPMD executor for the kernel program."""
    if "runner" in _STATE:
        return _STATE["runner"]
    import jax
    from jax.sharding import Mesh, PartitionSpec
    from jax.experimental.shard_map import shard_map
    from concourse import bass2jax

    nc = _build()
    bass2jax.install_neuronx_cc_hook()

    in_names, out_names, out_avals = [], [], []
    for alloc in nc.m.functions[0].allocations:
        if not isinstance(alloc, mybir.MemoryLocationSet):
            continue
        name = alloc.memorylocations[0].name
        pname = nc.partition_id_tensor.name if nc.partition_id_tensor else None
        if alloc.kind == "ExternalInput":
            if name != pname:
                in_names.append(name)
        elif alloc.kind == "ExternalOutput":
            out_names.append(name)
            out_avals.append(
                jax.core.ShapedArray(tuple(alloc.tensor_shape),
                                     mybir.dt.np(alloc.dtype))
            )
    n_params = len(in_names)
    all_in = list(in_names) + list(out_names)
    pname = nc.partition_id_tensor.name if nc.partition_id_tensor else None
    if pname is not None:
        all_in.append(pname)

    def _body(*args):
        operands = list(args)
        if pname is not None:
            operands.append(bass2jax.partition_id_tensor())
        outs = bass2jax._bass_exec_p.bind(
            *operands,
            out_avals=tuple(out_avals),
            in_names=tuple(all_in),
            out_names=tuple(out_names),
            lowering_input_output_aliases=(),
            sim_require_finite=False,
            sim_require_nnan=False,
            nc=nc,
        )
        return tuple(outs)

    devices = jax.devices()[:NCORES]
    mesh = Mesh(np.asarray(devices), ("core",))
    specs = (PartitionSpec("core"),)
    sharded = jax.jit(
        shard_map(
            _body, mesh=mesh,
            in_specs=specs * (n_params + len(out_names)),
            out_specs=specs * len(out_names),
            check_rep=False,
        ),
        keep_unused=True,
    )
    runner = {
        "fn": sharded, "in_names": in_names, "out_names": out_names,
        "out_avals": out_avals, "n_params": n_params,
    }
    _STATE["runner"] = runner
    return runner


def _prep_inputs(x, cos, sin, w_qkv, w_o):
    """Host-side sharding: per-core input dict list."""
    x = np.asarray(x, dtype=np.float32)
    cos = np.asarray(cos, dtype=np.float32)
    sin = np.asarray(sin, dtype=np.float32)
    w_qkv = np.asarray(w_qkv, dtype=np.float32)
    w_o = np.asarray(w_o, dtype=np.float32)

    bf = ml_dtypes.bfloat16
    cosT = np.ascontiguousarray(cos.T).astype(bf)           # [128, S]
    sinT = sin.T
    sinTs = np.ascontiguousarray(
        np.concatenate([-sinT[0:64], sinT[64:128]], axis=0)).astype(bf)
    pp, ff = np.meshgrid(np.arange(128), np.arange(128), indexing="ij")
    trimask = (pp <= ff).astype(np.float32)                 # t <= s

    in_maps = []
    for c in range(NCORES):
        b, tp = c // TPDEG, c % TPDEG
        cs = 512 * tp
        xT = np.ascontiguousarray(x[b].T)                   # [D, S]
        wq = w_qkv[:, cs:cs + 512]
        wk = w_qkv[:, D + cs:D + cs + 512]
        wqk = np.ascontiguousarray(np.concatenate([wq, wk], axis=1))
        wvs = np.ascontiguousarray(w_qkv[:, 2 * D + cs:2 * D + cs + 512])
        wos = np.ascontiguousarray(w_o[:, cs:cs + 512])
        in_maps.append({
            "xT": xT.astype(bf), "wqk": wqk.astype(bf), "wv": wvs.astype(bf),
            "wo": wos.astype(bf),
            "cosT": cosT, "sinTs": sinTs, "trimask": trimask.astype(bf),
        })
    return in_maps


def _run(in_maps):
    import jax
    r = _get_runner()
    concat = [
        np.concatenate([np.asarray(in_maps[c][n]) for c in range(NCORES)], axis=0)
        for n in r["in_names"]
    ]
    zeros = [
        np.zeros((NCORES * a.shape[0],) + tuple(a.shape[1:]), a.dtype)
        for a in r["out_avals"]
    ]
    outs = r["fn"](*concat, *zeros)
    outs = [np.asarray(o) for o in jax.block_until_ready(outs)]
    per_core = []
    for c in range(NCORES):
        d = {}
        for i, n in enumerate(r["out_names"]):
            shp = r["out_avals"][i].shape
            d[n] = outs[i].reshape((NCORES,) + shp)[c]
        per_core.append(d)
    return per_core


def kernel(x, cos, sin, w_qkv, w_o):
    in_maps = _prep_inputs(x, cos, sin, w_qkv, w_o)
    results = _run(in_maps)
    B = x.shape[0]
    out = np.empty((B, S, D), dtype=np.float32)
    for c in range(NCORES):
        b, tp = c // TPDEG, c % TPDEG
        out[b, :, 512 * tp:512 * tp + 512] = results[c]["out"]
    return out



# revision 20
# speedup vs baseline: 9.4331x; 1.0380x over previous
"""Trainium2 Bass kernel for nn_Attention_84756884619871.

Causal multi-head attention (B=2, S=2048, D=2048, H=16, Dh=128) with RoPE,
fused QKV projection and output projection.

Sharding (8 NeuronCores): data-parallel over batch (2 groups) x
tensor-parallel over heads (4 cores/group, 4 heads each).  Each core:
  - single pass over x^T in 512-column chunks computes q^T,k^T (RoPE
    applied via one ScalarE PSUM evacuation + three bf16 DVE ops) AND v
  - flash-style attention in score-transposed space (p^T[t,s]); softmax
    denominator via ones-vector matmul; no max-subtraction (scores are
    small: exp is safe in fp32)
  - attention runs per 512-column s-quarter, interleaved with the
    projection chunks (quarter q is emitted right after x-chunk q), so
    each of the four AllGathers launches as early as possible and
    overlaps the remaining projection/attention compute
  - the output projections for quarters 0-2 run while the final
    AllGather is in flight; only quarter 3's 64 matmuls trail it
  - DMA queue discipline: weight/x loads + agin stores on the SP HWDGE
    ring, output stores on the ACT ring, ctxg gathers on the Pool SWDGE
    queue, so no long-latency wait ever head-blocks another queue
All matmuls run in bf16 at N=512 (full PE rate, one PSUM bank each).
Host assembles the full [2,2048,2048] output from the 8 column slices.
"""

import numpy as np
import ml_dtypes

import concourse.bass as bass
import concourse.tile as tile
import concourse.mybir as mybir
from concourse import bacc
from concourse.tile import add_dep_helper
from contextlib import ExitStack

F32 = mybir.dt.float32
F32R = mybir.dt.bfloat16  # matmul operand dtype
AF = mybir.ActivationFunctionType

D = 2048
S = 2048
NCORES = 8
TPDEG = 4          # tensor-parallel group size (heads)
HLOC = 4           # heads per core
DH = 128
SCALE = float(1.0 / np.sqrt(DH))
NQ = 4             # s-quarters for attention / collectives
QW = S // NQ       # 512

_STATE: dict = {}
_PHASES: list = []


def _build(reps=1):
    _PHASES.clear()
    nc = bacc.Bacc("TRN2", target_bir_lowering=False, debug=False, num_devices=NCORES)
    xT = nc.dram_tensor("xT", [D, S], F32R, kind="ExternalInput")
    # host pre-tiles wqk to [ki, m, ko, c] so one m-column block (the unit
    # the q/k chains consume) is a single 4KB-per-partition contiguous DMA
    wqk = nc.dram_tensor("wqk", [128, 8 * 16 * 128], F32R, kind="ExternalInput")
    wv = nc.dram_tensor("wv", [D, 512], F32R, kind="ExternalInput")
    wo = nc.dram_tensor("wo", [D, 512], F32R, kind="ExternalInput")
    cosT = nc.dram_tensor("cosT", [128, S], F32R, kind="ExternalInput")
    sinTs = nc.dram_tensor("sinTs", [128, S], F32R, kind="ExternalInput")
    trimask = nc.dram_tensor("trimask", [128, 128], F32R, kind="ExternalInput")
    out = nc.dram_tensor("out", [S, 512], F32, kind="ExternalOutput")

    xT3 = xT.ap().rearrange("(ko ki) s -> ki ko s", ki=128)
    wqk4 = wqk.ap().rearrange("p (m ko c) -> p m ko c", m=8, ko=16)
    wv3 = wv.ap().rearrange("(ko ki) c -> ki ko c", ki=128)
    wo3 = wo.ap().rearrange("(ko ki) c -> ki ko c", ki=128)

    with tile.TileContext(nc) as tc, ExitStack() as top:
        # ---- persistent constants -----------------------------------------
        per = top.enter_context(tc.tile_pool(name="persist", bufs=1))
        mask_sb = per.tile([128, 128], F32R, name="mask")
        nc.sync.dma_start(mask_sb[:], trimask.ap())
        ones_c0 = per.tile([128, 128], F32, name="ones_c0")
        nc.vector.memset(ones_c0[:], 1.0)
        ones_sq = per.tile([128, 128], F32R, name="ones_sq")
        nc.vector.tensor_copy(ones_sq[:], ones_c0[:])

        dram = top.enter_context(tc.tile_pool(name="dram", bufs=1, space="DRAM"))
        agin = [dram.tile([HLOC * 128, QW], F32R, name=f"agin{q}") for q in range(NQ)]
        agout = [dram.tile([D, QW], F32R, name=f"agout{q}") for q in range(NQ)]

        qkv = top.enter_context(ExitStack())
        qk_pool = qkv.enter_context(tc.tile_pool(name="qkpool", bufs=1))
        qrot = [qk_pool.tile([128, S], F32R, name=f"qrot{h}") for h in range(HLOC)]
        krot = [qk_pool.tile([128, S], F32R, name=f"krot{h}") for h in range(HLOC)]
        v_pool = qkv.enter_context(tc.tile_pool(name="vpool", bufs=1))
        vsb = [v_pool.tile([128, 512], F32R, name=f"v{j}") for j in range(16)]

        wo_pool = top.enter_context(tc.tile_pool(name="wop", bufs=1))
        wo_sb = wo_pool.tile([128, 16, 512], F32R, name="wo_sb")

        for _rep in range(reps):
            _emit_rep(nc, tc, _rep, xT3, wqk4, wv3, wo3, cosT, sinTs, out,
                      mask_sb, ones_sq, agin, agout,
                      qrot, krot, vsb, wo_sb)

    _PHASES.append(('end', nc.next_id()))
    nc.compile()
    return nc


def _emit_rep(nc, tc, rep, xT3, wqk4, wv3, wo3, cosT, sinTs, out,
              mask_sb, ones_sq, agin, agout,
              qrot, krot, vsb, wo_sb):
    """One full kernel body: A-chunks (projection+RoPE+v) interleaved with
    attention quarters so each AllGather launches as early as possible;
    quarter 3 is woven with the earlier quarters' output projections."""
    last_pe = [None]

    with ExitStack() as outer:
        p_pool = outer.enter_context(tc.tile_pool(name="pp", bufs=4))
        misc = outer.enter_context(tc.tile_pool(name="miscb", bufs=2))

        def attn_quarter(sq, sc_ps, ctx_ps, l_ps, after_head=None):
            _PHASES.append((f'attn{sq}', nc.next_id()))
            for h in range(HLOC):
                ctx = ctx_ps.tile([128, QW], F32, tag="ctx", name=f"ctx{sq}_{h}")
                lps = l_ps.tile([128, QW], F32, tag="l", name=f"l{sq}_{h}")
                jmax = 4 * sq + 4
                for j in range(jmax):
                    dj = j - 4 * sq
                    c0 = max(0, 128 * dj)
                    sct = sc_ps.tile([128, QW], F32, tag="scps",
                                     name=f"sc{sq}_{h}_{j}")
                    nc.tensor.matmul(
                        sct[:, c0:QW], krot[h][:, 128 * j:128 * j + 128],
                        qrot[h][:, QW * sq + c0:QW * sq + QW],
                        start=True, stop=True,
                    )
                    p_t = p_pool.tile([128, QW], F32R, tag="p",
                                      name=f"p{sq}_{h}_{j}")
                    nc.scalar.activation(p_t[:, c0:QW], sct[:, c0:QW],
                                         AF.Exp, scale=SCALE)
                    if dj >= 0:
                        dsl = slice(c0, c0 + 128)
                        nc.vector.tensor_mul(p_t[:, dsl], p_t[:, dsl], mask_sb[:])
                    last = (j == jmax - 1)
                    pv_mm = nc.tensor.matmul(
                        ctx[:, c0:QW], vsb[j][:, 128 * h:128 * h + 128],
                        p_t[:, c0:QW], start=(j == 0), stop=last,
                        skip_group_check=True,
                    )
                    last_pe[0] = pv_mm
                    nc.tensor.matmul(
                        lps[:, c0:QW], ones_sq[:], p_t[:, c0:QW],
                        start=(j == 0), stop=last, skip_group_check=True,
                    )
                # normalize: ctxn = ctx * (1/l); the ones-128 stationary
                # already broadcast l across all partitions, so a plain
                # elementwise reciprocal+mul suffices (no PE broadcast)
                linv = misc.tile([128, QW], F32, tag="linv", name=f"li{sq}_{h}")
                nc.vector.reciprocal_approx_fast(out=linv[:], in_=lps[:])
                ctxn = misc.tile([128, QW], F32R, tag="ctxn", name=f"cn{sq}_{h}")
                nc.vector.tensor_mul(ctxn[:], ctx[:], linv[:])
                # SP-ring HWDGE: every A-phase load is emitted before any
                # attention, so the SP queue is drained by now; this keeps
                # the ctxn cross-engine wait off the ACT sequencer, which is
                # the bottleneck engine inside quarter 3
                nc.sync.dma_start(agin[sq][128 * h:128 * h + 128, :], ctxn[:])
                if after_head is not None:
                    after_head(h)
            _PHASES.append((f'AG{sq}', nc.next_id()))
            nc.gpsimd.collective_compute(
                "AllGather", mybir.AluOpType.bypass,
                ins=[agin[sq][:]], outs=[agout[sq][:]],
                replica_groups=[[0, 1, 2, 3], [4, 5, 6, 7]],
            )

        with ExitStack() as st:
            wqk_pool = st.enter_context(tc.tile_pool(name="wqkp", bufs=1))
            wqk_sb = wqk_pool.tile([128, 8, 16, 128], F32R, name="wqk_sb")
            wv_pool = st.enter_context(tc.tile_pool(name="wvp", bufs=1))
            wv_sb = wv_pool.tile([128, 16, 512], F32R, name="wv_sb")
            cs_pool = st.enter_context(tc.tile_pool(name="csp", bufs=1))
            cos_sb = cs_pool.tile([128, S], F32R, name="cos_sb")
            sin_sb = cs_pool.tile([128, S], F32R, name="sin_sb")
            xt_pool = st.enter_context(tc.tile_pool(name="xtp", bufs=3))
            xt_tiles = [None] * NQ

            def xt_dma(sc, part=None):
                if xt_tiles[sc] is None:
                    xt_tiles[sc] = xt_pool.tile([128, 16, QW], F32R, tag="xt",
                                                name=f"xt{sc}")
                t = xt_tiles[sc]
                ssl = slice(QW * sc, QW * sc + QW)
                if part is None:
                    nc.sync.dma_start(t[:], xT3[:, :, ssl])
                else:
                    nc.sync.dma_start(t[:, 4 * part:4 * part + 4, :],
                                      xT3[:, 4 * part:4 * part + 4, ssl])

            # startup-critical DMA order: first weight group + first x slab
            # land before the rest so the first matmuls can issue early.
            # wv/cos/sin go on the ACT ring (idle until the c-phase stores)
            # so the SP ring only carries the wqk+x bytes the first q/k
            # matmul chains are actually waiting on.
            xt_dma(0, 0)
            nc.sync.dma_start(wqk_sb[:, 0], wqk4[:, 0])
            for g in range(1, 4):
                xt_dma(0, g)
            for m in range(1, 8):
                nc.sync.dma_start(wqk_sb[:, m], wqk4[:, m])
            for g in range(4):
                nc.scalar.dma_start(wv_sb[:, 4 * g:4 * g + 4, :],
                                    wv3[:, 4 * g:4 * g + 4, :])
            nc.scalar.dma_start(cos_sb[:], cosT.ap())
            nc.scalar.dma_start(sin_sb[:], sinTs.ap())
            xt_dma(1)
            xt_dma(2)
            if rep == 0:
                for g in range(4):
                    nc.scalar.dma_start(wo_sb[:, 4 * g:4 * g + 4, :],
                                        wo3[:, 4 * g:4 * g + 4, :])

            rope_pool = st.enter_context(tc.tile_pool(name="ropep", bufs=2))
            ps_qk = st.enter_context(tc.tile_pool(name="psqk", bufs=2,
                                                  space="PSUM"))
            ps_v = st.enter_context(tc.tile_pool(name="psv", bufs=2,
                                                 space="PSUM"))
            sc_e = st.enter_context(tc.tile_pool(name="scpse", bufs=2,
                                                 space="PSUM"))
            ctx_e = st.enter_context(tc.tile_pool(name="ctxpse", bufs=1,
                                                  space="PSUM"))
            l_e = st.enter_context(tc.tile_pool(name="lpse", bufs=1,
                                                space="PSUM"))

            def a_chunk(sc):
                _PHASES.append((f'A:sc{sc}', nc.next_id()))
                sl = slice(QW * sc, QW * sc + QW)
                xt_c = xt_tiles[sc]
                # chunk 0 is DMA-paced (the 4MB wqk stream outruns the SP
                # ring): weave the v chains — fed by the parallel ACT-ring
                # wv load — between the first q/k chains so the PE has
                # work while the next wqk group lands
                morder = ([('m', 0), ('v', 0), ('m', 1), ('v', 1), ('m', 2),
                           ('v', 2), ('m', 3), ('v', 3), ('m', 4), ('m', 5),
                           ('m', 6), ('m', 7)] if sc == 0 else
                          [('m', m) for m in range(8)] +
                          [('v', u) for u in range(4)])
                for kind, idx in morder:
                    if kind == 'v':
                        a_chunk_v1(sc, idx)
                        continue
                    m = idx
                    pq = ps_qk.tile([128, QW], F32, tag="pq", name=f"pq{sc}_{m}")
                    for ko in range(16):
                        nc.tensor.matmul(pq[:],
                                         wqk_sb[:, m, ko, :],
                                         xt_c[:, ko, :],
                                         start=(ko == 0), stop=(ko == 15))
                    pqb = rope_pool.tile([128, QW], F32R, tag="pqb",
                                         name=f"pqb{sc}_{m}")
                    nc.scalar.copy(pqb[:], pq[:])
                    t1 = rope_pool.tile([128, QW], F32R, tag="t1",
                                        name=f"t1_{sc}_{m}")
                    nc.vector.tensor_mul(t1[:], pqb[:], cos_sb[:, sl])
                    t2 = rope_pool.tile([128, QW], F32R, tag="t2",
                                        name=f"t2_{sc}_{m}")
                    # rotate-half reads must come from PSUM: a partition-base
                    # mismatch between two SBUF operands is rejected by the
                    # BIR verifier, but PSUM+SBUF operands carry independent
                    # base-partition offsets
                    nc.vector.tensor_mul(t2[0:64, :], pq[64:128, :],
                                         sin_sb[0:64, sl])
                    nc.vector.tensor_mul(t2[64:128, :], pq[0:64, :],
                                         sin_sb[64:128, sl])
                    dest = (qrot[m] if m < 4 else krot[m - 4])[:, sl]
                    nc.vector.tensor_add(dest, t1[:], t2[:])

            def a_chunk_v1(sc, u):
                xt_c = xt_tiles[sc]
                j = 4 * sc + u
                pv = ps_v.tile([128, 512], F32, tag="pv", name=f"pv{j}")
                for ko in range(16):
                    nc.tensor.matmul(
                        pv[:], xt_c[:, ko, 128 * u:128 * u + 128],
                        wv_sb[:, ko, :], start=(ko == 0), stop=(ko == 15),
                    )
                nc.scalar.copy(vsb[j][:], pv[:])

            a_chunk(0)
            xt_dma(3)
            attn_quarter(0, sc_e, ctx_e, l_e)
            a_chunk(1)
            attn_quarter(1, sc_e, ctx_e, l_e)
            a_chunk(2)
            attn_quarter(2, sc_e, ctx_e, l_e)
            a_chunk(3)

        # ---- quarter 3 + phase C woven (A pools released) -----------------
        with ExitStack() as st:
            cg_pool = st.enter_context(tc.tile_pool(name="cgp", bufs=4))
            osb_pool = st.enter_context(tc.tile_pool(name="osbp", bufs=2))
            ps_o = st.enter_context(tc.tile_pool(name="pso", bufs=2,
                                                 space="PSUM"))
            sc_l = st.enter_context(tc.tile_pool(name="scpsl", bufs=3,
                                                 space="PSUM"))
            ctx_l = st.enter_context(tc.tile_pool(name="ctxpsl", bufs=2,
                                                  space="PSUM"))
            l_l = st.enter_context(tc.tile_pool(name="lpsl", bufs=1,
                                                space="PSUM"))
            ctxg = [None] * NQ

            def cg_load(sq):
                _PHASES.append((f'cg{sq}', nc.next_id()))
                cg = cg_pool.tile([128, 16, QW], F32R, tag="cg",
                                  name=f"ctxg{sq}")
                ag3 = agout[sq].rearrange("(ko ki) s -> ki ko s", ki=128)
                # SWDGE queue: must not head-block the HWDGE rings.
                # Split per ko-group so the c-block matmuls for group g
                # only wait on their own slice (pipelines the gather load
                # with the output projection instead of serializing).
                for g4 in range(4):
                    nc.gpsimd.dma_start(cg[:, 4 * g4:4 * g4 + 4, :],
                                        ag3[:, 4 * g4:4 * g4 + 4, :])
                ctxg[sq] = cg

            def c_block(sq, mms):
                _PHASES.append((f'c{sq}.{mms[0]}', nc.next_id()))
                anchor = last_pe[0]
                for mm in mms:
                    m = 4 * sq + mm
                    po = ps_o.tile([128, 512], F32, tag="pso", name=f"po{m}")
                    for ko in range(16):
                        o_mm = nc.tensor.matmul(
                            po[:], ctxg[sq][:, ko, 128 * mm:128 * mm + 128],
                            wo_sb[:, ko, :], start=(ko == 0), stop=(ko == 15),
                        )
                        if ko == 0 and anchor is not None:
                            # keep the scheduler from hoisting the output
                            # projection ahead of attention compute that
                            # does not depend on the AllGather
                            add_dep_helper(o_mm.ins, anchor.ins, sync=True,
                                           reason="order C after attention")
                        anchor = o_mm
                        last_pe[0] = o_mm
                    osb = osb_pool.tile([128, 512], F32, tag="osb",
                                        name=f"osb{m}")
                    nc.scalar.copy(osb[:], po[:])
                    # ACT ring: producer (scalar.copy) is on the same engine,
                    # so the store issues with no cross-engine wait and stays
                    # clear of the Pool queue that carries the cg loads
                    nc.scalar.dma_start(out.ap()[128 * m:128 * m + 128, :],
                                        osb[:])

            # cg3 is emitted late: its wait on the final AllGather must not
            # head-block the Pool queue in front of earlier quarters' traffic
            cg_load(0)
            cg_load(1)
            cg_load(2)

            attn_quarter(3, sc_l, ctx_l, l_l)
            c_block(0, [0, 1, 2, 3])
            c_block(1, [0, 1, 2, 3])
            c_block(2, [0, 1, 2, 3])
            cg_load(3)
            c_block(3, [0, 1, 2, 3])


def _get_runner():
    """Build (once) a persistent jitted SPMD executor for the kernel program."""
    if "runner" in _STATE:
        return _STATE["runner"]
    import jax
    from jax.sharding import Mesh, PartitionSpec
    from jax.experimental.shard_map import shard_map
    from concourse import bass2jax

    nc = _build()
    bass2jax.install_neuronx_cc_hook()

    in_names, out_names, out_avals = [], [], []
    for alloc in nc.m.functions[0].allocations:
        if not isinstance(alloc, mybir.MemoryLocationSet):
            continue
        name = alloc.memorylocations[0].name
        pname = nc.partition_id_tensor.name if nc.partition_id_tensor else None
        if alloc.kind == "ExternalInput":
            if name != pname:
                in_names.append(name)
        elif alloc.kind == "ExternalOutput":
            out_names.append(name)
            out_avals.append(
                jax.core.ShapedArray(tuple(alloc.tensor_shape),
                                     mybir.dt.np(alloc.dtype))
            )
    n_params = len(in_names)
    all_in = list(in_names) + list(out_names)
    pname = nc.partition_id_tensor.name if nc.partition_id_tensor else None
    if pname is not None:
        all_in.append(pname)

    def _body(*args):
        operands = list(args)
        if pname is not None:
            operands.append(bass2jax.partition_id_tensor())
        outs = bass2jax._bass_exec_p.bind(
            *operands,
            out_avals=tuple(out_avals),
            in_names=tuple(all_in),
            out_names=tuple(out_names),
            lowering_input_output_aliases=(),
            sim_require_finite=False,
            sim_require_nnan=False,
            nc=nc,
        )
        return tuple(outs)

    devices = jax.devices()[:NCORES]
    mesh = Mesh(np.asarray(devices), ("core",))
    specs = (PartitionSpec("core"),)
    sharded = jax.jit(
        shard_map(
            _body, mesh=mesh,
            in_specs=specs * (n_params + len(out_names)),
            out_specs=specs * len(out_names),
            check_rep=False,
        ),
        keep_unused=True,
    )
    runner = {
        "fn": sharded, "in_names": in_names, "out_names": out_names,
        "out_avals": out_avals, "n_params": n_params,
    }
    _STATE["runner"] = runner
    return runner


def _prep_inputs(x, cos, sin, w_qkv, w_o):
    """Host-side sharding: per-core input dict list."""
    x = np.asarray(x, dtype=np.float32)
    cos = np.asarray(cos, dtype=np.float32)
    sin = np.asarray(sin, dtype=np.float32)
    w_qkv = np.asarray(w_qkv, dtype=np.float32)
    w_o = np.asarray(w_o, dtype=np.float32)

    bf = ml_dtypes.bfloat16
    cosT = np.ascontiguousarray(cos.T).astype(bf)           # [128, S]
    sinT = sin.T
    sinTs = np.ascontiguousarray(
        np.concatenate([-sinT[0:64], sinT[64:128]], axis=0)).astype(bf)
    pp, ff = np.meshgrid(np.arange(128), np.arange(128), indexing="ij")
    trimask = (pp <= ff).astype(np.float32)                 # t <= s

    in_maps = []
    for c in range(NCORES):
        b, tp = c // TPDEG, c % TPDEG
        cs = 512 * tp
        xT = np.ascontiguousarray(x[b].T)                   # [D, S]
        wq = w_qkv[:, cs:cs + 512]
        wk = w_qkv[:, D + cs:D + cs + 512]
        wqk = np.concatenate([wq, wk], axis=1)
        # pre-tile [D, 1024] -> [ki, m, ko, c] so each 128-wide column
        # block lands via one contiguous-per-partition DMA at startup
        wqk = np.ascontiguousarray(
            wqk.reshape(16, 128, 8, 128).transpose(1, 2, 0, 3)
            .reshape(128, 8 * 16 * 128))
        wvs = np.ascontiguousarray(w_qkv[:, 2 * D + cs:2 * D + cs + 512])
        wos = np.ascontiguousarray(w_o[:, cs:cs + 512])
        in_maps.append({
            "xT": xT.astype(bf), "wqk": wqk.astype(bf), "wv": wvs.astype(bf),
            "wo": wos.astype(bf),
            "cosT": cosT, "sinTs": sinTs, "trimask": trimask.astype(bf),
        })
    return in_maps


def _run(in_maps):
    import jax
    r = _get_runner()
    concat = [
        np.concatenate([np.asarray(in_maps[c][n]) for c in range(NCORES)], axis=0)
        for n in r["in_names"]
    ]
    zeros = [
        np.zeros((NCORES * a.shape[0],) + tuple(a.shape[1:]), a.dtype)
        for a in r["out_avals"]
    ]
    outs = r["fn"](*concat, *zeros)
    outs = [np.asarray(o) for o in jax.block_until_ready(outs)]
    per_core = []
    for c in range(NCORES):
        d = {}
        for i, n in enumerate(r["out_names"]):
            shp = r["out_avals"][i].shape
            d[n] = outs[i].reshape((NCORES,) + shp)[c]
        per_core.append(d)
    return per_core


def kernel(x, cos, sin, w_qkv, w_o):
    in_maps = _prep_inputs(x, cos, sin, w_qkv, w_o)
    results = _run(in_maps)
    B = x.shape[0]
    out = np.empty((B, S, D), dtype=np.float32)
    for c in range(NCORES):
        b, tp = c // TPDEG, c % TPDEG
        out[b, :, 512 * tp:512 * tp + 512] = results[c]["out"]
    return out

